# revision 20
# baseline (speedup 1.0000x reference)
"""Trainium2 Bass kernel for the AttentionBlock problem (v2).

Full inputs:  x [16, 64, 64, 64] f32, w_theta [8, 64], w_phi [8, 64],
              w_g [32, 64], w_o [64, 32], gamma [] (all f32).
Sharding: data-parallel over batch, 2 samples per core on 8 NeuronCores.

Per-sample math (C=64, S=4096, T=S/4=1024):
  theta = w_theta @ x            [8, S]
  phi   = pool2x2(w_phi @ x)     [8, T]
  g     = pool2x2(w_g @ x)       [32, T]
  scoresT[t, s] = sum_c phi[c, t] theta[c, s]
  E = exp-approx(scoresT)  (mix of exact ACT exp -> bf16 and Schraudolph
      bit-trick on DVE: int16(2^7*log2e*s + 127*2^7) bitcast as bf16)
  g' = (gamma*w_o) @ g           [64, T]   (output 1x1 conv folded into V)
  attnU[s, c] = sum_t E[t, s] g'[c, t];  attnU[s, 64] = Z[s] = sum_t E[t, s]
     (fused attn+out-conv: E-tiles are the matmul STATIONARY [128t x 128s],
      moving rhs is g'T [128t, 65] incl. a ones column for Z)
  device output: attnU [S, 65] bf16 per sample
  host: out[c, s] = attnU[s, c] / attnU[s, 64] + x[c, s]

Engine budget per core (TimelineSim cost model):
  PE   ~46us: scores 65536 cols + attn 33280 + conv 8192 + g' 1024
  ACT  ~54us: conv-out copies, g'T staging, attnU staging, exp share
  DVE  ~54us: maxpools, Schraudolph share
"""

import sys

if "/opt/trn_rl_repo" not in sys.path:
    sys.path.insert(0, "/opt/trn_rl_repo")

import ml_dtypes
import numpy as np

import concourse.bass as bass
import concourse.tile as tile
from concourse import bacc, mybir
from concourse.bass_utils import run_bass_kernel_spmd

F32 = mybir.dt.float32
F32R = mybir.dt.float32r
BF16 = mybir.dt.bfloat16
I16 = mybir.dt.int16
AF = mybir.ActivationFunctionType
ALU = mybir.AluOpType

B, C, H, W = 16, 64, 64, 64
S = H * W            # 4096
T = S // 4           # 1024
NCORES = 8
BLOC = B // NCORES   # 2 samples per core
NT = T // 128        # 8 t-tiles
CHUNK = 512          # s-chunk size (one PSUM bank)
NCH = S // CHUNK     # 8 chunks per sample

A_SCH = 184.6649652337873    # 2^7 * log2(e)
B_SCH = 127.0 * 128.0        # 16256
# t-tiles handled by DVE (Schraudolph) vs ACT (exact exp), per chunk
DVE_TILES = (0, 2, 4, 6)

_EXT = {}


def _phase_a_steps(nc, tc, pools, s):
    """Build sample s's phase A (x load, conv, pools, g'T staging) as a list
    of emission thunks so it can be spread between chunk t-slots. Returns
    (handles, steps)."""
    (pp_sc, pp_at, pp_sm, p_samp, p_chunk, p_w) = pools

    # x layout: partition 64*a + c holds x[c, 2048*a + j] for j in [0,2048)
    x_sb = p_samp.tile([128, 2048], F32R, tag="x_sb", name=f"x_sb_{s}")
    tpg_sb = p_samp.tile([96, 4096], F32R, tag="tpg_sb", name=f"tpg_sb_{s}")
    phi_sb = p_samp.tile([8, T], F32R, tag="phi_sb", name=f"phi_sb_{s}")
    g_sb = p_samp.tile([32, T], BF16, tag="g_sb", name=f"g_sb_{s}")
    pwf_sb = p_samp.tile([8, 2048], F32R, tag="pwf_sb", name=f"pwf_sb_{s}")
    pwg_sb = p_samp.tile([32, 2048], BF16, tag="pwg_sb", name=f"pwg_sb_{s}")
    gt_sb = p_samp.tile([128, NT * 65], BF16, tag="gt_sb", name=f"gt_sb_{s}")

    steps = []

    def dma_x(q):
        # q=0: first half (A); q=2: second half (B); big transfers amortize
        # the per-DMA fixed costs (HWDGE slot + DGE delay + sem prop)
        if s == 0 and q == 0:
            # weights first, then a small x slice so conv(0) starts ASAP
            nc.sync.dma_start(_EXT["wct_sb"][:], _EXT["wct"][:])
            nc.sync.dma_start(x_sb[0:64, 0:512], _EXT["x"][s, :, 0:512])
            nc.sync.dma_start(x_sb[0:64, 512:2048], _EXT["x"][s, :, 512:2048])
            nc.sync.dma_start(_EXT["wogt_sb"][:], _EXT["wogt"][:])
            return
        if q == 0:
            nc.sync.dma_start(x_sb[0:64, :], _EXT["x"][s, :, 0:2048])
        elif q == 2:
            nc.sync.dma_start(x_sb[64:128, :], _EXT["x"][s, :, 2048:4096])
        # q==1,3: covered by the q==0/q==2 transfers

    # conv chunk k covers s in [512k, 512k+512); out rows:
    # theta 0:8, phi 32:40, g 64:96
    def conv(k):
        a = k // 4
        ps_conv = pp_sm.tile([96, 512], F32, tag="sm", name=f"ps_conv_{s}_{k}")
        nc.tensor.matmul(
            ps_conv[:],
            _EXT["wct_sb"][64 * a:64 * a + 64, :],
            x_sb[64 * a:64 * a + 64, (k % 4) * 512:(k % 4) * 512 + 512],
            start=True, stop=True,
        )
        nc.scalar.activation(tpg_sb[:, k * 512:(k + 1) * 512], ps_conv[:],
                             AF.Copy)

    def pool_w(dst, src):
        sv = src.rearrange("p (x two) -> p x two", two=2)
        dv = dst.rearrange("p (x one) -> p x one", one=1)
        nc.vector.tensor_max(dv, sv[:, :, 0:1], sv[:, :, 1:2])

    def pool_h(dst, src):
        # src [p, 2048] = 64 h-rows x 32 w; pairs of h-rows -> [p, 1024]
        sv = src.rearrange("p (q r w) -> p q r w", r=2, w=32)
        dv = dst.rearrange("p (q one w) -> p q one w", one=1, w=32)
        nc.vector.tensor_max(dv, sv[:, :, 0:1, :], sv[:, :, 1:2, :])

    def pools_half(h, which):
        sl = slice(h * 2048, (h + 1) * 2048)
        tl = slice(h * 1024, (h + 1) * 1024)
        if which == 0:
            pool_w(pwf_sb[:, tl], tpg_sb[32:40, sl])
            pool_h(phi_sb[:, h * 512:(h + 1) * 512],
                   pwf_sb.rearrange("p (h q) -> p h q", h=2)[:, h, :])
        else:
            pool_w(pwg_sb[:, tl], tpg_sb[64:96, sl])
            pool_h(g_sb[:, h * 512:(h + 1) * 512],
                   pwg_sb.rearrange("p (h q) -> p h q", h=2)[:, h, :])

    gv = gt_sb.rearrange("p (t c) -> p t c", c=65)
    ps_gp = [None]

    def gprime(half):
        # g'T tiles [128t, 65] bf16 (col 64 = ones), 8 tiles in one psum bank
        if half == 0:
            ps_gp[0] = pp_sm.tile([128, 512], F32, tag="sm", name=f"ps_gp_{s}")
        for t in range(4 * half, 4 * half + 4):
            nc.tensor.matmul(
                ps_gp[0][:, 64 * t:64 * t + 64],
                g_sb[:, t * 128:(t + 1) * 128],
                _EXT["wogt_sb"][:],
                start=True, stop=True,
            )

    def gstage():
        nc.scalar.activation(gv[:, :, 0:64],
                             ps_gp[0].rearrange("p (t c) -> p t c", c=64)[:],
                             AF.Copy)
        nc.vector.memset(gv[:, :, 64:65], 1.0)

    steps.append(lambda: (dma_x(0), dma_x(1)))
    steps.append(lambda: (dma_x(2), dma_x(3)))
    for k in range(8):
        steps.append(lambda k=k: conv(k))
        if k == 3:
            steps.append(lambda: pools_half(0, 0))
            steps.append(lambda: pools_half(0, 1))
    steps.append(lambda: pools_half(1, 0))
    steps.append(lambda: pools_half(1, 1))
    steps.append(lambda: gprime(0))
    steps.append(lambda: gprime(1))
    steps.append(gstage)

    return (x_sb, tpg_sb, phi_sb, gt_sb), steps


def _emit_chunk(nc, pools, s, ch, handles, prev, phase_a_step=None):
    """Software-pipelined chunk body.

    Emits scores+exp for chunk (s, ch) with the attn matmuls of the PREVIOUS
    chunk interleaved per t-tile (so PE never runs a long attn-only stretch
    while the exp engines starve), then staging+DMA of the previous chunk.
    prev = (s', ch', e_tiles', gv') or None. Returns this chunk's (s, ch,
    e_tiles, gv). phase_a_step optionally emits a slice of the next sample's
    phase A between t-tiles.
    """
    (pp_sc, pp_at, pp_sm, p_samp, p_chunk, p_w) = pools

    ps_at = av_ = flush_sv = None
    if prev is not None:
        ps_, ch_, e_tiles_, gv_ = prev
        ps_at = pp_at.tile([128, 260], F32, tag="at", name=f"ps_at_{ps_}_{ch_}")
        av_ = ps_at.rearrange("p (j c) -> p j c", c=65)

    e_tiles = []
    if ch is not None:
        x_sb, tpg_sb, phi_sb, gt_sb = handles[s]
        theta = tpg_sb[0:8, :]
        gv = gt_sb.rearrange("p (t c) -> p t c", c=65)

    for t in range(NT):
        if ch is not None:
            ps_sc = pp_sc.tile([128, CHUNK], F32, tag="sc",
                               name=f"ps_sc_{s}_{ch}_{t}")
            nc.tensor.matmul(
                ps_sc[:],
                phi_sb[:, t * 128:(t + 1) * 128],
                theta[:, ch * CHUNK:(ch + 1) * CHUNK],
                start=True, stop=True,
            )
            if t in DVE_TILES:
                e16 = p_chunk.tile([128, CHUNK], I16, tag=f"e{t}",
                                   name=f"e16_{s}_{ch}_{t}", bufs=4)
                nc.vector.tensor_scalar(e16[:], ps_sc[:], A_SCH, B_SCH,
                                        ALU.mult, ALU.add)
                e_tiles.append(e16.bitcast(BF16))
            else:
                ebf = p_chunk.tile([128, CHUNK], BF16, tag=f"e{t}",
                                   name=f"ebf_{s}_{ch}_{t}", bufs=4)
                nc.scalar.activation(ebf[:], ps_sc[:], AF.Exp)
                e_tiles.append(ebf)
        if prev is not None and t % 2 == 1:
            # previous chunk's attn: j-group (t-1)//2 as one contiguous
            # accumulation run over all 8 t-tiles
            j = (t - 1) // 2
            for tt in range(NT):
                nc.tensor.matmul(
                    av_[:, j, :],
                    e_tiles_[tt][:, j * 128:(j + 1) * 128],
                    gv_[:, tt, :],
                    start=(tt == 0), stop=(tt == NT - 1),
                )
        if phase_a_step is not None:
            phase_a_step(t)

    if prev is not None:
        stg = p_chunk.tile([128, 260], BF16, tag="stg",
                           name=f"stg_{ps_}_{ch_}", bufs=3)
        sv = stg.rearrange("p (j c) -> p j c", c=65)
        if ch is None:
            nc.vector.tensor_copy(stg[:], ps_at[:])
        else:
            nc.scalar.activation(stg[:], ps_at[:], AF.Copy)
        nc.sync.dma_start(_EXT["out"][ps_, ch_, :, :, :], sv[:])

    if ch is None:
        return None
    return (s, ch, e_tiles, gv)


def build_nc():
    nc = bacc.Bacc("TRN2", target_bir_lowering=False, debug=False,
                   num_devices=NCORES)
    _EXT["x"] = nc.dram_tensor("x", [BLOC, C, S], F32R, kind="ExternalInput").ap()
    _EXT["wct"] = nc.dram_tensor("wct", [128, 96], F32R, kind="ExternalInput").ap()
    _EXT["wogt"] = nc.dram_tensor("wogt", [32, 64], BF16, kind="ExternalInput").ap()
    # out[s, ch, p, j, c]: query index = 512*ch + 128*j + p; c in [0,65)
    _EXT["out"] = nc.dram_tensor("out", [BLOC, NCH, 128, 4, 65], BF16,
                                 kind="ExternalOutput").ap()

    with tile.TileContext(nc) as tc:
        with (
            tc.tile_pool(name="wpool", bufs=1) as p_w,
            tc.tile_pool(name="samp", bufs=2) as p_samp,
            tc.tile_pool(name="chunk", bufs=2) as p_chunk,
            tc.tile_pool(name="ppsc", bufs=4, space="PSUM") as pp_sc,
            tc.tile_pool(name="ppat", bufs=2, space="PSUM") as pp_at,
            tc.tile_pool(name="ppsm", bufs=2, space="PSUM") as pp_sm,
        ):
            _EXT["wct_sb"] = p_w.tile([128, 96], F32R, tag="wct_sb", name="wct_sb")
            _EXT["wogt_sb"] = p_w.tile([32, 64], BF16, tag="wogt_sb", name="wogt_sb")

            pools = (pp_sc, pp_at, pp_sm, p_samp, p_chunk, p_w)
            handles = [None] * BLOC
            handles[0], steps0 = _phase_a_steps(nc, tc, pools, 0)
            for st in steps0:
                st()
            # sample 1's phase A is spread across sample 0's chunks 1..5,
            # one step every other t-slot, so its conv/pool/copy work fills
            # engine slack instead of stalling the chunk pipeline
            step_q = []
            prev = None
            for s in range(BLOC):
                for ch in range(NCH):
                    if s == 0 and ch == 1 and s + 1 < BLOC:
                        handles[s + 1], steps1 = _phase_a_steps(nc, tc, pools,
                                                                s + 1)
                        step_q = list(steps1)

                    def pa_step(t):
                        if step_q and t % 2 == 1:
                            step_q.pop(0)()

                    prev = _emit_chunk(nc, pools, s, ch, handles, prev,
                                       phase_a_step=pa_step if step_q else None)
            while step_q:
                step_q.pop(0)()
            _emit_chunk(nc, pools, None, None, handles, prev)

    nc.compile()
    return nc


_NC_CACHE = None


def _get_nc():
    global _NC_CACHE
    if _NC_CACHE is None:
        _NC_CACHE = build_nc()
    return _NC_CACHE


def kernel(x, w_theta, w_phi, w_g, w_o, gamma):
    x = np.ascontiguousarray(np.asarray(x, dtype=np.float32))
    w_theta = np.asarray(w_theta, dtype=np.float32)
    w_phi = np.asarray(w_phi, dtype=np.float32)
    w_g = np.asarray(w_g, dtype=np.float32)
    w_o = np.asarray(w_o, dtype=np.float32)
    gamma_f = float(np.asarray(gamma, dtype=np.float32))

    # conv lhsT [64, 96]: theta.T at cols 0:8, phi.T at 32:40, g.T at 64:96,
    # replicated on partitions 64:128 for the second x half
    wcat = np.zeros((64, 96), dtype=np.float32)
    wcat[:, 0:8] = w_theta.T
    wcat[:, 32:40] = w_phi.T
    wcat[:, 64:96] = w_g.T
    wct = np.tile(wcat, (2, 1))                                  # [128, 96]
    wogt = np.ascontiguousarray((gamma_f * w_o).T).astype(ml_dtypes.bfloat16)

    nc = _get_nc()
    xr = x.reshape(B, C, S)
    in_maps = [
        {
            "x": np.ascontiguousarray(xr[i * BLOC:(i + 1) * BLOC]),
            "wct": wct,
            "wogt": wogt,
        }
        for i in range(NCORES)
    ]
    res = run_bass_kernel_spmd(nc, in_maps, core_ids=list(range(NCORES)))
    # res: [BLOC, NCH, 4, 128, 65] bf16 per core -> attnU [B, S, 65]
    au = np.concatenate(
        [np.asarray(res.results[i]["out"]) for i in range(NCORES)], axis=0
    ).astype(np.float32)                       # [B, NCH, 128, 4, 65]
    au = au.transpose(0, 1, 3, 2, 4).reshape(B, S, 65)
    o = au[:, :, 0:64] / au[:, :, 64:65]                          # [B, S, 64]
    out = o.transpose(0, 2, 1) + xr                               # [B, C, S]
    return np.ascontiguousarray(out.reshape(B, C, H, W).astype(np.float32))


if __name__ == "__main__":
    rng = np.random.default_rng(0)
    ins = {
        "x": rng.standard_normal((B, C, H, W), dtype=np.float32),
        "w_theta": (rng.standard_normal((8, 64)) / 8.0).astype(np.float32),
        "w_phi": (rng.standard_normal((8, 64)) / 8.0).astype(np.float32),
        "w_g": (rng.standard_normal((32, 64)) / 8.0).astype(np.float32),
        "w_o": (rng.standard_normal((64, 32)) / np.sqrt(32)).astype(np.float32),
        "gamma": np.float32(0.7),
    }
    out = kernel(**ins)
    print("out", out.shape, out.dtype, np.abs(out).mean())
